# revision 1
# baseline (speedup 1.0000x reference)
"""Trainium2 Bass kernel for nn_Agent2Agent_emb (gnn_message_passing).

Reference computes, for each batch b:
    edge[b,m,n,e] = pairwise features of (agent1[b,m], agent2[b,n])   (E=8)
    out[b,m,n,h]  = einsum("mne,he->mnh", edge, W) + bias             (H=128)

Every edge feature is bilinear in per-m and per-n quantities, so the whole
output factors exactly as a rank-5 product

    out[b,m,n,h] = sum_{k<5} P[b,m,k] * R[b,k,n,h]

with P built from agent1 rows and R built from agent2 rows, W and bias
(see _build_factors).  The device kernel is then a tiny-K matmul that
expands [5 x N1] x [5 x (N2*H)] per batch -- pure memory-bound output
streaming, which matches the target regime.

For speed the matmul runs in bf16 with an hi/lo error-compensated split
(out ~= Phi@Rhi + Phi@Rlo + Plo@Rhi, K=15 padded to 16), giving ~1e-5
relative error vs the fp32 reference while streaming at 1 column/cycle.

Sharding: one batch element per NeuronCore (B == n_cores == 8); each core
writes its own [N1, N2*H] slab, gathered on host by np.stack.
"""

import numpy as np
import ml_dtypes

B, N1, N2, D, E, H = 8, 256, 256, 7, 8, 128
XY_SCALE = 10.0
NCORES = 8
K = 16          # contraction dim on device (15 live rows + 1 zero pad)
FDIM = N2 * H   # 32768, flattened (n, h) free dim

# device tiling
OCH = 4096      # sbuf output-staging chunk (per-partition elements)
PCH = 1024      # psum tile free size (2 fp32 banks, 1 concurrent matmul pair)
MM = 512        # free dim per matmul

# The device computes and stores the output in fp16 (upcast to fp32 on the
# host).  Output rounding gives ~4.9e-4 relative error, far below the bf16
# noise level the reference family tolerates, and halves the HBM store
# traffic that bounds this memory-regime kernel.
OUT_DT = "float16"

NCH = FDIM // OCH      # 8 column chunks
NR = 4                 # r-slot ring depth
NO = 8                 # output-staging ring depth
TILES_PER_CHUNK = 2 * (OCH // PCH)  # 2 mc x 4 fi = 8
NTILES = NCH * TILES_PER_CHUNK      # 64
NMM = NTILES * (PCH // MM)          # 128

# copy-engine assignment per psum tile: ScalarE is ~9% faster per element,
# so it takes 34 of the 64 copies (odd tiles plus tiles 0 and 32)
_ENG = ["s" if (T % 2 == 1 or T in (0, 32)) else "v" for T in range(NTILES)]
_CV_PRE = [sum(1 for t in range(T + 1) if _ENG[t] == "v") for T in range(NTILES)]
_CS_PRE = [sum(1 for t in range(T + 1) if _ENG[t] == "s") for T in range(NTILES)]

_BF16 = ml_dtypes.bfloat16


def _build_factors(agent1, agent2, W, b):
    """Host-side rank-5 factorization + bf16 hi/lo split.

    Returns AT [B, K, N1] bf16 (matmul lhsT) and RR [B, K, FDIM] bf16
    (matmul rhs), with row layout [Phi(5) | Phi(5) | Plo(5) | 0] and
    [Rhi(5) | Rlo(5) | Rhi(5) | 0] so that lhsT.T @ rhs reproduces
    Phi@Rhi + Phi@Rlo + Plo@Rhi.
    """
    a1_f32 = np.asarray(agent1)
    a2_f32 = np.asarray(agent2)
    a1 = a1_f32.astype(np.float64)
    a2 = a2_f32.astype(np.float64)
    Wd = np.asarray(W).astype(np.float64)
    bd = np.asarray(b).astype(np.float64)

    f1 = (~np.all(a1_f32 == 0, axis=-1)).astype(np.float64)  # [B,N1]
    f2 = (~np.all(a2_f32 == 0, axis=-1)).astype(np.float64)  # [B,N2]

    x1x, x1y, s1, c1 = a1[..., 0], a1[..., 1], a1[..., 3], a1[..., 4]
    x2x, x2y, v2, s2, c2 = a2[..., 0], a2[..., 1], a2[..., 2], a2[..., 3], a2[..., 4]

    # m-side basis P [B, N1, 5]
    P = np.stack(
        [
            f1 * c1,
            f1 * s1,
            -f1 * (c1 * x1x + s1 * x1y),
            f1 * (s1 * x1x - c1 * x1y),
            np.ones_like(f1),
        ],
        axis=-1,
    )

    # n-side basis g [B, N2]
    g1 = f2 * x2x
    g2 = f2 * x2y
    g3 = f2
    g4 = f2 * s2
    g5 = f2 * c2
    g6 = f2 * s2 * v2
    g7 = f2 * c2 * v2
    g8 = a2[..., 5]
    g9 = a2[..., 6]

    s = XY_SCALE
    W0, W1, W2, W3, W4, W5, W6, W7 = (Wd[:, e] for e in range(8))

    def outer(g, w):  # [B,N2] x [H] -> [B,N2,H]
        return g[..., None] * w[None, None, :]

    R1 = (
        outer(g1, W0) / s
        + outer(g2, W1) / s
        + outer(g4, W2)
        + outer(g5, W3)
        + outer(g6, W4)
        + outer(g7, W5)
    )
    R2 = (
        outer(g2, W0) / s
        - outer(g1, W1) / s
        - outer(g5, W2)
        + outer(g4, W3)
        - outer(g7, W4)
        + outer(g6, W5)
    )
    R3 = outer(g3, W0) / s
    R4 = outer(g3, W1) / s
    R5 = outer(g8, W6) + outer(g9, W7) + bd[None, None, :]
    R = np.stack([R1, R2, R3, R4, R5], axis=1)  # [B, 5, N2, H]

    Phi = P.astype(_BF16)
    Plo = (P - Phi.astype(np.float64)).astype(_BF16)
    Rhi = R.astype(_BF16)
    Rlo = (R - Rhi.astype(np.float64)).astype(_BF16)

    PhiT = Phi.transpose(0, 2, 1)  # [B, 5, N1]
    PloT = Plo.transpose(0, 2, 1)

    AT = np.zeros((B, K, N1), dtype=_BF16)
    AT[:, 0:5] = PhiT
    AT[:, 5:10] = PhiT
    AT[:, 10:15] = PloT

    RR = np.zeros((B, K, FDIM), dtype=_BF16)
    Rhif = Rhi.reshape(B, 5, FDIM)
    Rlof = Rlo.reshape(B, 5, FDIM)
    RR[:, 0:5] = Rhif
    RR[:, 5:10] = Rlof
    RR[:, 10:15] = Rhif
    return AT, RR


def build_bass():
    import concourse.mybir as mybir
    from concourse import bacc
    from contextlib import ExitStack

    nc = bacc.Bacc()
    out_dt = getattr(mybir.dt, OUT_DT)
    atr = nc.dram_tensor("atr", [K, N1], mybir.dt.bfloat16, kind="ExternalInput")
    rr = nc.dram_tensor("rr", [K, FDIM], mybir.dt.bfloat16, kind="ExternalInput")
    out = nc.dram_tensor("out", [N1, FDIM], out_dt, kind="ExternalOutput")

    ctx = ExitStack()
    with ctx:
        at_sb = ctx.enter_context(nc.sbuf_tensor("at_sb", [48, N1], mybir.dt.bfloat16))
        r_sb = [
            ctx.enter_context(
                nc.sbuf_tensor(f"r_sb{i}", [48, OCH], mybir.dt.bfloat16)
            )
            for i in range(NR)
        ]
        ot_sb = [
            ctx.enter_context(nc.sbuf_tensor(f"ot_sb{i}", [128, OCH], out_dt))
            for i in range(NO)
        ]
        ps = [
            ctx.enter_context(
                nc.psum_tensor(f"ps{i}", [128, PCH], mybir.dt.float32)
            )
            for i in range(4)
        ]
        s_at = ctx.enter_context(nc.semaphore("s_at"))
        s_r0a = ctx.enter_context(nc.semaphore("s_r0a"))
        s_rs = [ctx.enter_context(nc.semaphore(f"s_r{i}")) for i in range(NR)]
        s_mm = ctx.enter_context(nc.semaphore("s_mm"))
        s_cv = ctx.enter_context(nc.semaphore("s_cv"))
        s_cs = ctx.enter_context(nc.semaphore("s_cs"))
        s_sts = [ctx.enter_context(nc.semaphore(f"s_st{i}")) for i in range(NO)]
        block = ctx.enter_context(nc.Block())

        def tile_info(T):
            j = T // TILES_PER_CHUNK
            mc = (T // (OCH // PCH)) % 2
            fi = T % (OCH // PCH)
            return j, mc, fi

        class WaitTracker:
            """Skip waits already implied by earlier waits on this engine."""

            def __init__(self, eng):
                self.eng = eng
                self.seen = {}

            def wait(self, sem, val):
                key = id(sem)
                if self.seen.get(key, -1) >= val:
                    return
                self.seen[key] = val
                self.eng.wait_ge(sem, val)

        def copy_body(eng, which, inc_sem):
            w = WaitTracker(eng)
            for T in range(NTILES):
                if _ENG[T] != which:
                    continue
                j, mc, fi = tile_info(T)
                O = T // (OCH // PCH)
                w.wait(s_mm, 2 * (T + 1))
                if O >= NO:
                    w.wait(s_sts[O % NO], 16 * (O // NO))
                dst = ot_sb[O % NO][:, fi * PCH : (fi + 1) * PCH]
                if which == "v":
                    eng.tensor_copy(dst, ps[T % 4][:]).then_inc(inc_sem, 1)
                else:
                    eng.copy(dst, ps[T % 4][:]).then_inc(inc_sem, 1)

        @block.scalar
        def _(scalar):
            scalar.dma_start(at_sb[0:K, :], atr[:]).then_inc(s_at, 16)
            scalar.dma_start(at_sb[32 : 32 + K, :], atr[:]).then_inc(s_at, 16)
            copy_body(scalar, "s", s_cs)

        @block.vector
        def _(vector):
            copy_body(vector, "v", s_cv)

        # chunk 0 is loaded in two column pieces so the PE can start on the
        # first PCH columns while the rest streams in
        C0 = PCH

        def r_full_val(j):
            # s_rs[j % NR] value once chunk j is fully loaded
            return 32 * (j // NR + 1)

        @block.gpsimd
        def _(gpsimd):
            w = WaitTracker(gpsimd)
            for j in range(NCH):
                if j >= NR:
                    w.wait(s_mm, 2 * TILES_PER_CHUNK * (j - NR + 1))
                sl = r_sb[j % NR]
                src = rr[:, j * OCH : (j + 1) * OCH]
                if j == 0:
                    # first PCH columns on a dedicated sem so fi=0 matmuls
                    # can start before the rest of the chunk lands
                    gpsimd.dma_start(sl[0:K, :C0], src[:, :C0]).then_inc(
                        s_r0a, 16
                    )
                    gpsimd.dma_start(
                        sl[32 : 32 + K, :C0], src[:, :C0]
                    ).then_inc(s_r0a, 16)
                    gpsimd.dma_start(sl[0:K, C0:], src[:, C0:]).then_inc(
                        s_rs[0], 16
                    )
                    gpsimd.dma_start(
                        sl[32 : 32 + K, C0:], src[:, C0:]
                    ).then_inc(s_rs[0], 16)
                else:
                    gpsimd.dma_start(sl[0:K, :], src).then_inc(s_rs[j % NR], 16)
                    gpsimd.dma_start(sl[32 : 32 + K, :], src).then_inc(
                        s_rs[j % NR], 16
                    )

        @block.tensor
        def _(tensor):
            w = WaitTracker(tensor)
            w.wait(s_at, 32)
            for i in range(NMM):
                T = i // 2
                g = i % 2
                j, mc, fi = tile_info(T)
                if j == 0 and fi == 0:
                    w.wait(s_r0a, 32)  # first PCH columns of chunk 0
                else:
                    w.wait(s_rs[j % NR], r_full_val(j))
                if g == 0 and T >= 4:
                    P = T - 4
                    if _ENG[P] == "v":
                        w.wait(s_cv, _CV_PRE[P])
                    else:
                        w.wait(s_cs, _CS_PRE[P])
                base = 32 * g
                lo = fi * PCH + g * MM
                tensor.matmul(
                    ps[T % 4][:, g * MM : (g + 1) * MM],
                    at_sb[base : base + K, mc * 128 : (mc + 1) * 128],
                    r_sb[j % NR][base : base + K, lo : lo + MM],
                    start=True,
                    stop=True,
                ).then_inc(s_mm, 1)

        @block.sync
        def _(sync):
            w = WaitTracker(sync)
            for S in range(NCH * 2):
                j = S // 2
                mc = S % 2
                T_last = 4 * S + 3
                w.wait(s_cv, _CV_PRE[T_last])
                w.wait(s_cs, _CS_PRE[T_last])
                sync.dma_start(
                    out[mc * 128 : (mc + 1) * 128, j * OCH : (j + 1) * OCH],
                    ot_sb[S % NO][:],
                ).then_inc(s_sts[S % NO], 16)

    nc.compile()
    return nc


_NC_CACHE = None


def _get_nc():
    global _NC_CACHE
    if _NC_CACHE is None:
        _NC_CACHE = build_bass()
    return _NC_CACHE


def run(agent1, agent2, W, b, trace=False):
    from concourse.bass_utils import run_bass_kernel_spmd

    AT, RR = _build_factors(agent1, agent2, W, b)
    in_maps = [
        {"atr": np.ascontiguousarray(AT[c]), "rr": np.ascontiguousarray(RR[c])}
        for c in range(NCORES)
    ]
    res = run_bass_kernel_spmd(
        _get_nc(), in_maps, core_ids=list(range(NCORES)), trace=trace
    )
    out = np.stack(
        [
            np.asarray(res.results[c]["out"]).astype(np.float32).reshape(N1, N2, H)
            for c in range(NCORES)
        ]
    )
    return out, res


def kernel(agent1, agent2, W, b):
    out, _ = run(agent1, agent2, W, b, trace=False)
    return out

